# revision 11
# baseline (speedup 1.0000x reference)
"""Blended-expert MLP (MoE routing) Trainium2 Bass kernel.

Math: reference computes, per layer,
    h = elu( einsum("bi,bio->bo", x, einsum("be,eio->bio", c, w)) + c @ b )
which factorizes as
    h = elu( sum_e (c[:,e] * x) @ W_e  +  c @ b )
(row-scaling commutes with the matmul), so per layer we scale X^T by
c_e on the vector engine and run 8 [rows,512]x[512,512] matmuls plus
one tiny K=8 matmul for the blended bias, ALL accumulating into a
single PSUM tile. Then ELU, then a PE transpose to produce the next
layer's stationary operand.

Sharding: data-parallel over the batch. B=512 rows split across 8
NeuronCores (64 rows each); the expert weights are replicated to every
core. No collectives (on-chip AllReduce has a ~20us latency floor).

int8 weight path (the big lever vs the fp16 baseline): the weight DMA
is the wall (fp16 = 12.6 MB/core at ~358 GB/s/core = 35 us). Weights
are uniform-distributed, so int8 quantization costs only ~0.4%%/layer
(measured 6.8e-3 total rel err vs the 2e-2 budget) and halves the DMA
to 6.3 MB (~18.6 us). The PE has no int8 mode, so weights are widened
on-chip to EXACT fp16 integers in [-127,127]:
  - DVE chunks ride as uint16 byte-pairs (host packs col j of experts
    0-3 into the low byte, col j of experts 4-7 into the high byte);
    two tensor_scalar ops split them: (v & 0xFF) - 128 and
    (v >> 8) - 128. All operands 2-byte -> DVE 4x mode, ~1.2us/chunk.
  - ACT / GPSIMD chunks ride as plain uint8; one activation
    Copy(in - 128) / tensor_scalar subtract per half, ~3.5us/chunk.
  Assignment per layer: k=0,3 -> DVE (layer closers need low latency),
  k=1 -> GPSIMD, k=2 -> ACT.
The float scale s_w (per layer, max|w|/127) is applied at PSUM
evacuation time via a per-partition fp32 scale AP (runtime data, so
the NEFF needs no recompile if inputs change); biases are pre-divided
by s_w on the host (they are structurally zero in this problem).

fp16 x/matmul path unchanged from the baseline: ~7e-4 of the error
budget. PE warmup matmuls get the HAM clock gate to 2.4 GHz before
real work; k-outer matmul order fires each chunk's matmuls on widen
completion; even/odd experts run concurrently in the two column
halves of the PE array.
"""

import numpy as np

B, E, D = 512, 8, 512
NCORES = 8
ROWS = B // NCORES  # 64
KC = D // 128  # 4 contraction chunks of 128
NCHUNK = 3 * KC  # 12

# pack tensor column layout (per 128 partitions)
PK_XT = 0  # [128, 256]: layer-1 x^T chunk k at cols [64k, 64k+64)
PK_CB = 256  # [128, 1024]: c broadcast; col 128e+64j+b = C[b,e], all partitions
PK_ID = PK_CB + E * 2 * ROWS  # [64, 64]: identity, partitions 0..63
PK_CT = PK_ID + ROWS  # [8, 64]: coef^T, partitions 0..7
PCK = PK_CT + ROWS

MODE = "i8"
N_WARMUP = 7

# every chunk rides as uint16 byte-pairs: lo byte = experts 0-3, hi byte =
# experts 4-7. DVE extracts the hi half (arith, 4x mode); the lo half is a
# stride-2 uint8 view handled by ACT / GPSIMD (cost is free-size based):
LO_ENG = {0: "gps", 1: "act", 2: "gps", 3: "act"}
# matmul issue order: hi-half experts first (their extract lands first)
ORD_E = [4, 5, 6, 7, 0, 1, 2, 3]

_NC_CACHE = {}


def _build(mode):
    from contextlib import ExitStack

    import concourse.bacc as bacc
    import concourse.mybir as mybir
    import concourse.tile as tile

    f32 = mybir.dt.float32
    f16 = mybir.dt.float16
    u16 = mybir.dt.uint16
    u8 = mybir.dt.uint8
    Alu = mybir.AluOpType
    Act = mybir.ActivationFunctionType

    nc = bacc.Bacc()
    pack_d = nc.declare_dram_parameter("pack", [128, PCK], f16, isOutput=False)
    scl_d = nc.declare_dram_parameter("scl", [128, 4], f32, isOutput=False)
    bias_d = nc.declare_dram_parameter("biasd", [E, 3 * D], f16, isOutput=False)
    wpair_d = nc.declare_dram_parameter(
        "wpair", [NCHUNK, 128, E * D // 2], u16, isOutput=False
    )
    out_d = nc.declare_dram_parameter("out", [ROWS, D], f32, isOutput=True)

    HED = E * D // 2  # 2048: wfp column offset of experts 4-7

    with ExitStack() as ctx:
        tc = ctx.enter_context(tile.TileContext(nc))
        const = ctx.enter_context(tc.tile_pool(name="const", bufs=1))
        prpool = ctx.enter_context(tc.tile_pool(name="prp", bufs=4))
        wfpool = ctx.enter_context(tc.tile_pool(name="wfp", bufs=5))
        spool = ctx.enter_context(tc.tile_pool(name="sp", bufs=24))
        hpool = ctx.enter_context(tc.tile_pool(name="hp", bufs=2))
        xpool = ctx.enter_context(tc.tile_pool(name="xp", bufs=2))
        acc_ps = ctx.enter_context(tc.tile_pool(name="acc", bufs=3, space="PSUM"))
        pt_ps = ctx.enter_context(tc.tile_pool(name="pt", bufs=3, space="PSUM"))
        wm_ps = ctx.enter_context(tc.tile_pool(name="wm", bufs=1, space="PSUM"))

        # PE warmup: garbage matmuls on a zeroed tile (output never read) so
        # the HAM clock gate reaches 2.4 GHz before the first real matmul.
        # memset on DVE (ready ~3.3us) rather than gpsimd (ready later).
        warm = const.tile([128, ROWS + D], f16)
        nc.vector.memset(warm[:], 0.0)
        wps = wm_ps.tile([ROWS, D], f32, tag="warm")
        for _ in range(N_WARMUP):
            nc.tensor.matmul(
                wps[:], warm[:, 0:ROWS], warm[:, ROWS:], start=True, stop=True
            )

        scl_t = const.tile([128, 4], f32)
        nc.sync.dma_start(scl_t[:], scl_d[:])
        pack_t = const.tile([128, PCK], f16)
        nc.sync.dma_start(pack_t[:], pack_d[:])
        bias_t = const.tile([E, 3 * D], f16)
        nc.gpsimd.dma_start(bias_t[:], bias_d[:])

        coeft_ap = pack_t[0:E, PK_CT : PK_CT + ROWS]
        ident_ap = pack_t[0:ROWS, PK_ID : PK_ID + ROWS]
        xt_tile, xt_off = pack_t, PK_XT  # current x^T source: [128, 256] at offset

        # all weight-chunk DMAs up-front on the sync queue in consumption
        # order; the HWDGE lane round-robin paces them at full bandwidth
        raw_tiles = []
        for c in range(NCHUNK):
            wt = prpool.tile([128, HED], u16, tag="wp")
            if c == NCHUNK - 1:
                # split the final chunk so experts 0-1/4-5 land earlier and
                # only the tail experts gate on the very last transfer
                nc.sync.dma_start(
                    wt[:, 0 : HED // 2], wpair_d[c, :, 0 : HED // 2]
                )
                nc.sync.dma_start(wt[:, HED // 2 :], wpair_d[c, :, HED // 2 :])
            else:
                nc.sync.dma_start(wt[:], wpair_d[c, :, :])
            raw_tiles.append(wt)

        def widen(c):
            """Emit widen ops for chunk c; returns the fp16 weight tile."""
            wt = raw_tiles[c]
            wf = wfpool.tile([128, E * D], f16, tag="wf")
            # hi extract on DVE, all-arith (the ISA has no DVE mod/bitwise
            # mixing): v/256 - 128 = w_q_hi + lo_byte/256; the host pre-
            # compensates that leakage into the hi byte.
            nsplit = 2 if c == NCHUNK - 1 else 1
            step = HED // nsplit
            for s in range(nsplit):
                lo, hi = s * step, (s + 1) * step
                nc.vector.tensor_scalar(
                    wf[:, HED + lo : HED + hi], wt[:, lo:hi],
                    1.0 / 256.0, 128.0, Alu.mult, Alu.subtract,
                )
            # lo extract: stride-2 uint8 view minus 128, on ACT or GPSIMD
            # (their cost depends on free size, not stride or dtype)
            lov = wt[:].bitcast(u8).rearrange("p (n two) -> p two n", two=2)
            for s in range(nsplit):
                lo, hi = s * step, (s + 1) * step
                if LO_ENG[c % KC] == "act":
                    nc.scalar.activation(
                        wf[:, lo:hi], lov[:, 0, lo:hi], Act.Copy, bias=-128.0
                    )
                else:
                    nc.gpsimd.tensor_scalar(
                        wf[:, lo:hi], lov[:, 0, lo:hi], 128.0, None,
                        Alu.subtract,
                    )
            return wf

        for layer in range(3):
            # widen this layer's chunks (emitted per layer so queued engine
            # work stays roughly in execution order)
            wfs = [widen(layer * KC + k) for k in range(KC)]
            sw_ap = scl_t[0:ROWS, layer : layer + 1]

            # scale x^T by c_e along the batch (free) dim, per half so the
            # layer boundary pipelines at chunk-pair granularity
            scaled = []
            for e in range(E):
                sc = spool.tile([128, KC * ROWS], f16, tag="sc")
                for half in range(2):
                    lo, hi = 2 * ROWS * half, 2 * ROWS * (half + 1)
                    nc.vector.tensor_tensor(
                        out=sc[:, lo:hi],
                        in0=xt_tile[:, xt_off + lo : xt_off + hi],
                        in1=pack_t[
                            :,
                            PK_CB + 2 * ROWS * e : PK_CB + 2 * ROWS * (e + 1),
                        ],
                        op=Alu.mult,
                    )
                scaled.append(sc)

            # one accumulation group: 32 expert matmuls + bias matmul (K=8).
            # k-outer order: each chunk's 8 expert matmuls fire as soon as
            # its widen lands. Even/odd experts run CONCURRENTLY in the two
            # column halves of the PE array (tile_position); the partition
            # halves of acc are summed afterwards.
            acc = acc_ps.tile([2 * ROWS, D], f32, tag="acc")
            nc.tensor.matmul(
                acc[0:ROWS, :],
                coeft_ap,
                bias_t[:, D * layer : D * (layer + 1)],
                start=True,
                stop=False,
                tile_position=(0, 0),
                skip_group_check=True,
            )
            for k in range(KC):
                for e in ORD_E:
                    half = e % 2
                    nc.tensor.matmul(
                        acc[half * ROWS : (half + 1) * ROWS, :],
                        scaled[e][:, ROWS * k : ROWS * (k + 1)],
                        wfs[k][:, D * e : D * (e + 1)],
                        start=(k == 0 and e == ORD_E[1]),
                        stop=(k == KC - 1 and e in ORD_E[-2:]),
                        tile_position=(0, half * ROWS),
                        skip_group_check=True,
                    )

            # evacuate even half with the s_w scale (ACT), merge+scale the
            # odd half (DVE stt), elu, transpose; pipelined per 128-column
            # quarter so each quarter flows through the chain independently
            t0 = hpool.tile([ROWS, D], f32, tag="t0")
            hpre = hpool.tile([ROWS, D], f32, tag="hpre")
            HD = D // 2
            if layer < 2:
                ex = hpool.tile([ROWS, D], f32, tag="ex")
                h = hpool.tile([ROWS, D], f16, tag="h")
                xt_t = xpool.tile([128, KC * ROWS], f16, tag="xt")
                for q in range(KC):
                    qs = slice(128 * q, 128 * (q + 1))
                    nc.scalar.activation(
                        t0[:, qs], acc[0:ROWS, qs], Act.Copy, scale=sw_ap
                    )
                    nc.vector.scalar_tensor_tensor(
                        out=hpre[:, qs],
                        in0=acc[ROWS:, qs],
                        scalar=sw_ap,
                        in1=t0[:, qs],
                        op0=Alu.mult,
                        op1=Alu.add,
                    )
                    nc.scalar.activation(ex[:, qs], hpre[:, qs], Act.Exp)
                    nc.vector.tensor_scalar(
                        ex[:, qs], ex[:, qs], 1.0, 0.0, Alu.subtract, Alu.min
                    )
                    nc.vector.scalar_tensor_tensor(
                        out=h[:, qs],
                        in0=hpre[:, qs],
                        scalar=0.0,
                        in1=ex[:, qs],
                        op0=Alu.max,
                        op1=Alu.add,
                    )
                    pt = pt_ps.tile([128, ROWS], f16, tag="pt")
                    nc.tensor.transpose(pt[:], h[:, qs], ident_ap)
                    dst = xt_t[:, ROWS * q : ROWS * (q + 1)]
                    if q % 2 == 0:
                        nc.scalar.copy(dst, pt[:])
                    else:
                        nc.vector.tensor_copy(dst, pt[:])
                xt_tile, xt_off = xt_t, 0
            else:
                # stream the output per column half, right behind the merge
                for cc in range(2):
                    cs = slice(HD * cc, HD * (cc + 1))
                    nc.scalar.activation(
                        t0[:, cs], acc[0:ROWS, cs], Act.Copy, scale=sw_ap
                    )
                    nc.vector.scalar_tensor_tensor(
                        out=hpre[:, cs],
                        in0=acc[ROWS:, cs],
                        scalar=sw_ap,
                        in1=t0[:, cs],
                        op0=Alu.mult,
                        op1=Alu.add,
                    )
                    nc.sync.dma_start(out_d[:, cs], hpre[:, cs])

    nc.compile()
    return nc


def _get_nc(mode=MODE):
    if mode not in _NC_CACHE:
        _NC_CACHE[mode] = _build(mode)
    return _NC_CACHE[mode]


def _prep_in_maps(inputs, mode=MODE):
    X = np.asarray(inputs["X"], np.float32)
    C = np.asarray(inputs["blending_coef"], np.float32)
    ws = [np.asarray(inputs[f"w_l{i}"], np.float32) for i in (1, 2, 3)]
    bs = [np.asarray(inputs[f"b_l{i}"], np.float32) for i in (1, 2, 3)]

    # W[l][i, e*D+o] = w_l[e, i, o]; int8-quantize per layer
    sw = np.array([max(np.abs(w).max() / 127.0, 1e-30) for w in ws], np.float32)
    scaled_ws = []
    for l, w in enumerate(ws):
        W = w.transpose(1, 0, 2).reshape(D, E * D)
        scaled_ws.append((W / sw[l]).astype(np.float64))  # in [-127, 127]
    wpair = np.zeros((NCHUNK, 128, E * D // 2), np.uint16)
    HED = E * D // 2
    for c in range(NCHUNK):
        l, k = c // KC, c % KC
        sub = scaled_ws[l][128 * k : 128 * (k + 1)]
        # pair layout: lo byte = experts 0-3, hi byte = experts 4-7. The
        # on-chip hi extract is v/256 - 128 = w_hi + lo/256, so pre-
        # subtract the known lo/256 leakage before rounding.
        a = (np.round(sub[:, :HED]).clip(-127, 127) + 128.0).astype(np.uint16)
        b = np.round(sub[:, HED:] + 128.0 - a / 256.0).clip(0, 255)
        wpair[c] = a | (b.astype(np.uint16) << 8)

    Bb = np.concatenate([b / s for b, s in zip(bs, sw)], axis=1).astype(
        np.float16
    )  # [E, 3*D], pre-divided by s_w (zeros in this problem)
    scl = np.broadcast_to(
        np.concatenate([sw, [1.0]]).astype(np.float32), (128, 4)
    ).copy()

    in_maps = []
    for c in range(NCORES):
        rs = slice(c * ROWS, (c + 1) * ROWS)
        pack = np.zeros((128, PCK), np.float32)
        # xt chunks: pack[p, 64k+b] = X[rows][b, 128k+p]
        xt = np.ascontiguousarray(X[rs].T)  # [512, 64]
        pack[:, PK_XT : PK_XT + KC * ROWS] = (
            xt.reshape(KC, 128, ROWS).transpose(1, 0, 2).reshape(128, KC * ROWS)
        )
        # c broadcast: pack[p, PK_CB + 128e + 64j + b] = C[rs][b, e]
        pack[:, PK_CB : PK_CB + E * 2 * ROWS] = np.broadcast_to(
            C[rs].T[:, None, :], (E, 2, ROWS)
        ).reshape(1, E * 2 * ROWS)
        pack[0:ROWS, PK_ID : PK_ID + ROWS] = np.eye(ROWS, dtype=np.float32)
        pack[0:E, PK_CT : PK_CT + ROWS] = C[rs].T
        in_maps.append(
            {
                "pack": pack.astype(np.float16),
                "biasd": Bb,
                "wpair": wpair,
                "scl": scl,
            }
        )
    return in_maps


def run(inputs, mode=MODE, trace=False):
    """Returns (output [512,512] fp32, BassKernelResults)."""
    from concourse.bass_utils import run_bass_kernel_spmd

    nc = _get_nc(mode)
    in_maps = _prep_in_maps(inputs, mode)
    res = run_bass_kernel_spmd(nc, in_maps, list(range(NCORES)), trace=trace)
    out = np.concatenate([r["out"] for r in res.results], axis=0)
    return out, res


def kernel(**inputs) -> np.ndarray:
    out, _ = run(inputs)
    return out


# revision 16
# speedup vs baseline: 3.6107x; 3.6107x over previous
"""Blended-expert MLP (MoE routing) Trainium2 Bass kernel.

Math: reference computes, per layer,
    h = elu( einsum("bi,bio->bo", x, einsum("be,eio->bio", c, w)) + c @ b )
which factorizes as
    h = elu( sum_e (c[:,e] * x) @ W_e  +  c @ b )
(row-scaling commutes with the matmul), so per layer we scale X^T by
c_e on the vector engine and run 8 [rows,512]x[512,512] matmuls plus
one tiny K=8 matmul for the blended bias, ALL accumulating into a
single PSUM tile. Then ELU, then a PE transpose to produce the next
layer's stationary operand.

Sharding: data-parallel over the batch. B=512 rows split across 8
NeuronCores (64 rows each); the expert weights are replicated to every
core. No collectives (on-chip AllReduce has a ~20us latency floor).

int8 weight path (the big lever vs the fp16 baseline): the weight DMA
is the wall (fp16 = 12.6 MB/core at ~358 GB/s/core = 35 us). Weights
are uniform-distributed, so int8 quantization costs only ~0.4%%/layer
(measured 6.8e-3 total rel err vs the 2e-2 budget) and halves the DMA
to 6.3 MB (~18.6 us). The PE has no int8 mode, so weights are widened
on-chip to EXACT fp16 integers in [-127,127]:
  - DVE chunks ride as uint16 byte-pairs (host packs col j of experts
    0-3 into the low byte, col j of experts 4-7 into the high byte);
    two tensor_scalar ops split them: (v & 0xFF) - 128 and
    (v >> 8) - 128. All operands 2-byte -> DVE 4x mode, ~1.2us/chunk.
  - ACT / GPSIMD chunks ride as plain uint8; one activation
    Copy(in - 128) / tensor_scalar subtract per half, ~3.5us/chunk.
  Assignment per layer: k=0,3 -> DVE (layer closers need low latency),
  k=1 -> GPSIMD, k=2 -> ACT.
The float scale s_w (per layer, max|w|/127) is applied at PSUM
evacuation time via a per-partition fp32 scale AP (runtime data, so
the NEFF needs no recompile if inputs change); biases are pre-divided
by s_w on the host (they are structurally zero in this problem).

fp16 x/matmul path unchanged from the baseline: ~7e-4 of the error
budget. PE warmup matmuls get the HAM clock gate to 2.4 GHz before
real work; k-outer matmul order fires each chunk's matmuls on widen
completion; even/odd experts run concurrently in the two column
halves of the PE array.
"""

import numpy as np

B, E, D = 512, 8, 512
NCORES = 8
ROWS = B // NCORES  # 64
KC = D // 128  # 4 contraction chunks of 128
NCHUNK = 3 * KC  # 12

# pack tensor column layout (per 128 partitions)
PK_XT = 0  # [128, 256]: layer-1 x^T chunk k at cols [64k, 64k+64)
PK_CB = 256  # [128, 1024]: c broadcast; col 128e+64j+b = C[b,e], all partitions
PK_ID = PK_CB + E * 2 * ROWS  # [64, 64]: identity, partitions 0..63
PK_CT = PK_ID + ROWS  # [8, 64]: coef^T, partitions 0..7
PCK = PK_CT + ROWS

MODE = "i8"
N_WARMUP = 7

# every chunk rides as uint16 byte-pairs: lo byte = experts 0-3, hi byte =
# experts 4-7. DVE extracts the hi half via v/256-128 (693ns, 4x mode); the
# lo half is a stride-2 uint8 subtract: DVE 1.23us / ACT 2.0us measured.
# GPSIMD is catastrophic on integer ops (29us!) but fine on fp16 (800ns per
# [128,256] mult), so it takes most of the xs scaling instead.
LO_ENG = {0: "dve", 1: "act", 2: "act", 3: "act"}
# matmul issue order: hi-half experts first (their extract lands first)
ORD_E = [4, 5, 6, 7, 0, 1, 2, 3]
# xs scale ops: layers 1-2 keep the first-consumed experts on the fast DVE
DVE_SCALE_E = (4, 5, 6)

_NC_CACHE = {}


def _build(mode):
    from contextlib import ExitStack

    import concourse.bacc as bacc
    import concourse.mybir as mybir
    import concourse.tile as tile

    f32 = mybir.dt.float32
    f16 = mybir.dt.float16
    u16 = mybir.dt.uint16
    u8 = mybir.dt.uint8
    Alu = mybir.AluOpType
    Act = mybir.ActivationFunctionType

    nc = bacc.Bacc()
    pack_d = nc.declare_dram_parameter("pack", [128, PCK], f16, isOutput=False)
    scl_d = nc.declare_dram_parameter("scl", [128, 4], f32, isOutput=False)
    bias_d = nc.declare_dram_parameter("biasd", [E, 3 * D], f16, isOutput=False)
    wpair_d = nc.declare_dram_parameter(
        "wpair", [NCHUNK, 128, E * D // 2], u16, isOutput=False
    )
    out_d = nc.declare_dram_parameter("out", [ROWS, D], f32, isOutput=True)

    HED = E * D // 2  # 2048: wfp column offset of experts 4-7

    with ExitStack() as ctx:
        tc = ctx.enter_context(tile.TileContext(nc))
        const = ctx.enter_context(tc.tile_pool(name="const", bufs=1))
        prpool = ctx.enter_context(tc.tile_pool(name="prp", bufs=4))
        wfpool = ctx.enter_context(tc.tile_pool(name="wfp", bufs=5))
        spool = ctx.enter_context(tc.tile_pool(name="sp", bufs=24))
        hpool = ctx.enter_context(tc.tile_pool(name="hp", bufs=2))
        xpool = ctx.enter_context(tc.tile_pool(name="xp", bufs=2))
        acc_ps = ctx.enter_context(tc.tile_pool(name="acc", bufs=3, space="PSUM"))
        pt_ps = ctx.enter_context(tc.tile_pool(name="pt", bufs=3, space="PSUM"))
        wm_ps = ctx.enter_context(tc.tile_pool(name="wm", bufs=1, space="PSUM"))

        # PE warmup: garbage matmuls on a zeroed tile (output never read) so
        # the HAM clock gate reaches 2.4 GHz before the first real matmul.
        # memset on DVE (ready ~3.3us) rather than gpsimd (ready later).
        warm = const.tile([128, ROWS + D], f16)
        nc.vector.memset(warm[:], 0.0)
        wps = wm_ps.tile([ROWS, D], f32, tag="warm")
        for _ in range(N_WARMUP):
            nc.tensor.matmul(
                wps[:], warm[:, 0:ROWS], warm[:, ROWS:], start=True, stop=True
            )

        scl_t = const.tile([128, 4], f32)
        nc.sync.dma_start(scl_t[:], scl_d[:])
        pack_t = const.tile([128, PCK], f16)
        nc.sync.dma_start(pack_t[:], pack_d[:])
        bias_t = const.tile([E, 3 * D], f16)
        nc.gpsimd.dma_start(bias_t[:], bias_d[:])

        coeft_ap = pack_t[0:E, PK_CT : PK_CT + ROWS]
        ident_ap = pack_t[0:ROWS, PK_ID : PK_ID + ROWS]
        xt_tile, xt_off = pack_t, PK_XT  # current x^T source: [128, 256] at offset

        # all weight-chunk DMAs up-front on the sync queue in consumption
        # order; the HWDGE lane round-robin paces them at full bandwidth
        raw_tiles = []
        for c in range(NCHUNK):
            wt = prpool.tile([128, HED], u16, tag="wp")
            if c == NCHUNK - 1:
                # split the final chunk so experts 0-1/4-5 land earlier and
                # only the tail experts gate on the very last transfer
                nc.sync.dma_start(
                    wt[:, 0 : HED // 2], wpair_d[c, :, 0 : HED // 2]
                )
                nc.sync.dma_start(wt[:, HED // 2 :], wpair_d[c, :, HED // 2 :])
            else:
                nc.sync.dma_start(wt[:], wpair_d[c, :, :])
            raw_tiles.append(wt)

        def widen(c):
            """Emit widen ops for chunk c; returns the fp16 weight tile."""
            wt = raw_tiles[c]
            wf = wfpool.tile([128, E * D], f16, tag="wf")
            # hi extract on DVE, all-arith (the ISA has no DVE mod/bitwise
            # mixing): v/256 - 128 = w_q_hi + lo_byte/256; the host pre-
            # compensates that leakage into the hi byte.
            nsplit = 2 if c == NCHUNK - 1 else 1
            step = HED // nsplit
            for s in range(nsplit):
                lo, hi = s * step, (s + 1) * step
                nc.vector.tensor_scalar(
                    wf[:, HED + lo : HED + hi], wt[:, lo:hi],
                    1.0 / 256.0, 128.0, Alu.mult, Alu.subtract,
                )
            # lo extract: stride-2 uint8 view minus 128, on ACT or DVE
            lov = wt[:].bitcast(u8).rearrange("p (n two) -> p two n", two=2)
            for s in range(nsplit):
                lo, hi = s * step, (s + 1) * step
                if LO_ENG[c % KC] == "act":
                    nc.scalar.activation(
                        wf[:, lo:hi], lov[:, 0, lo:hi], Act.Copy, bias=-128.0
                    )
                else:
                    nc.vector.tensor_scalar(
                        wf[:, lo:hi], lov[:, 0, lo:hi], 128.0, None,
                        Alu.subtract,
                    )
            return wf

        for layer in range(3):
            # widen this layer's chunks (emitted per layer so queued engine
            # work stays roughly in execution order)
            wfs = [widen(layer * KC + k) for k in range(KC)]
            sw_ap = scl_t[0:ROWS, layer : layer + 1]

            # scale x^T by c_e along the batch (free) dim, one full-width op
            # per expert, emitted in consumption order. Layer 0 scales only
            # need pack, so the otherwise-idle GPSIMD does them all; later
            # layers keep the first-consumed experts on the faster DVE.
            scaled = [None] * E
            for e in ORD_E:
                sc = spool.tile([128, KC * ROWS], f16, tag="sc")
                eng = (
                    nc.vector
                    if layer > 0 and e in DVE_SCALE_E
                    else nc.gpsimd
                )
                cb = (
                    pack_t[:, PK_CB + 2 * ROWS * e : PK_CB + 2 * ROWS * (e + 1)]
                    .unsqueeze(1)
                    .broadcast_to((128, 2, 2 * ROWS))
                )
                eng.tensor_tensor(
                    out=sc[:].rearrange("p (r c) -> p r c", r=2),
                    in0=xt_tile[
                        :, xt_off : xt_off + KC * ROWS
                    ].rearrange("p (r c) -> p r c", r=2),
                    in1=cb,
                    op=Alu.mult,
                )
                scaled[e] = sc

            # one accumulation group: 32 expert matmuls + bias matmul (K=8).
            # k-outer order: each chunk's 8 expert matmuls fire as soon as
            # its widen lands. Even/odd experts run CONCURRENTLY in the two
            # column halves of the PE array (tile_position); the partition
            # halves of acc are summed afterwards.
            acc = acc_ps.tile([2 * ROWS, D], f32, tag="acc")
            nc.tensor.matmul(
                acc[0:ROWS, :],
                coeft_ap,
                bias_t[:, D * layer : D * (layer + 1)],
                start=True,
                stop=False,
                tile_position=(0, 0),
                skip_group_check=True,
            )
            for k in range(KC):
                for e in ORD_E:
                    half = e % 2
                    nc.tensor.matmul(
                        acc[half * ROWS : (half + 1) * ROWS, :],
                        scaled[e][:, ROWS * k : ROWS * (k + 1)],
                        wfs[k][:, D * e : D * (e + 1)],
                        start=(k == 0 and e == ORD_E[1]),
                        stop=(k == KC - 1 and e in ORD_E[-2:]),
                        tile_position=(0, half * ROWS),
                        skip_group_check=True,
                    )

            # evacuate even half with the s_w scale (ACT), merge+scale the
            # odd half (DVE stt), elu, transpose; pipelined per 128-column
            # quarter so each quarter flows through the chain independently
            t0 = hpool.tile([ROWS, D], f32, tag="t0")
            hpre = hpool.tile([ROWS, D], f32, tag="hpre")
            HD = D // 2
            if layer < 2:
                ex = hpool.tile([ROWS, D], f32, tag="ex")
                h = hpool.tile([ROWS, D], f16, tag="h")
                xt_t = xpool.tile([128, KC * ROWS], f16, tag="xt")
                # merge/elu per 256-col half (fewer, bigger DVE/ACT ops);
                # transpose + evacuation still per 128-col quarter
                for hh in range(2):
                    hs = slice(HD * hh, HD * (hh + 1))
                    nc.scalar.activation(
                        t0[:, hs], acc[0:ROWS, hs], Act.Copy, scale=sw_ap
                    )
                    nc.vector.scalar_tensor_tensor(
                        out=hpre[:, hs],
                        in0=acc[ROWS:, hs],
                        scalar=sw_ap,
                        in1=t0[:, hs],
                        op0=Alu.mult,
                        op1=Alu.add,
                    )
                    nc.scalar.activation(ex[:, hs], hpre[:, hs], Act.Exp)
                    nc.vector.tensor_scalar(
                        ex[:, hs], ex[:, hs], 1.0, 0.0, Alu.subtract, Alu.min
                    )
                    nc.vector.scalar_tensor_tensor(
                        out=h[:, hs],
                        in0=hpre[:, hs],
                        scalar=0.0,
                        in1=ex[:, hs],
                        op0=Alu.max,
                        op1=Alu.add,
                    )
                    for q in (2 * hh, 2 * hh + 1):
                        qs = slice(128 * q, 128 * (q + 1))
                        pt = pt_ps.tile([128, ROWS], f16, tag="pt")
                        nc.tensor.transpose(pt[:], h[:, qs], ident_ap)
                        dst = xt_t[:, ROWS * q : ROWS * (q + 1)]
                        if q % 2 == 0:
                            nc.scalar.copy(dst, pt[:])
                        else:
                            nc.vector.tensor_copy(dst, pt[:])
                xt_tile, xt_off = xt_t, 0
            else:
                # stream the output per column half, right behind the merge
                for cc in range(2):
                    cs = slice(HD * cc, HD * (cc + 1))
                    nc.scalar.activation(
                        t0[:, cs], acc[0:ROWS, cs], Act.Copy, scale=sw_ap
                    )
                    nc.vector.scalar_tensor_tensor(
                        out=hpre[:, cs],
                        in0=acc[ROWS:, cs],
                        scalar=sw_ap,
                        in1=t0[:, cs],
                        op0=Alu.mult,
                        op1=Alu.add,
                    )
                    nc.sync.dma_start(out_d[:, cs], hpre[:, cs])

    nc.compile()
    return nc


def _get_nc(mode=MODE):
    if mode not in _NC_CACHE:
        _NC_CACHE[mode] = _build(mode)
    return _NC_CACHE[mode]


def _prep_in_maps(inputs, mode=MODE):
    X = np.asarray(inputs["X"], np.float32)
    C = np.asarray(inputs["blending_coef"], np.float32)
    ws = [np.asarray(inputs[f"w_l{i}"], np.float32) for i in (1, 2, 3)]
    bs = [np.asarray(inputs[f"b_l{i}"], np.float32) for i in (1, 2, 3)]

    # W[l][i, e*D+o] = w_l[e, i, o]; int8-quantize per layer
    sw = np.array([max(np.abs(w).max() / 127.0, 1e-30) for w in ws], np.float32)
    scaled_ws = []
    for l, w in enumerate(ws):
        W = w.transpose(1, 0, 2).reshape(D, E * D)
        scaled_ws.append((W / sw[l]).astype(np.float64))  # in [-127, 127]
    wpair = np.zeros((NCHUNK, 128, E * D // 2), np.uint16)
    HED = E * D // 2
    for c in range(NCHUNK):
        l, k = c // KC, c % KC
        sub = scaled_ws[l][128 * k : 128 * (k + 1)]
        # pair layout: lo byte = experts 0-3, hi byte = experts 4-7. The
        # on-chip hi extract is v/256 - 128 = w_hi + lo/256, so pre-
        # subtract the known lo/256 leakage before rounding.
        a = (np.round(sub[:, :HED]).clip(-127, 127) + 128.0).astype(np.uint16)
        b = np.round(sub[:, HED:] + 128.0 - a / 256.0).clip(0, 255)
        wpair[c] = a | (b.astype(np.uint16) << 8)

    Bb = np.concatenate([b / s for b, s in zip(bs, sw)], axis=1).astype(
        np.float16
    )  # [E, 3*D], pre-divided by s_w (zeros in this problem)
    scl = np.broadcast_to(
        np.concatenate([sw, [1.0]]).astype(np.float32), (128, 4)
    ).copy()

    in_maps = []
    for c in range(NCORES):
        rs = slice(c * ROWS, (c + 1) * ROWS)
        pack = np.zeros((128, PCK), np.float32)
        # xt chunks: pack[p, 64k+b] = X[rows][b, 128k+p]
        xt = np.ascontiguousarray(X[rs].T)  # [512, 64]
        pack[:, PK_XT : PK_XT + KC * ROWS] = (
            xt.reshape(KC, 128, ROWS).transpose(1, 0, 2).reshape(128, KC * ROWS)
        )
        # c broadcast: pack[p, PK_CB + 128e + 64j + b] = C[rs][b, e]
        pack[:, PK_CB : PK_CB + E * 2 * ROWS] = np.broadcast_to(
            C[rs].T[:, None, :], (E, 2, ROWS)
        ).reshape(1, E * 2 * ROWS)
        pack[0:ROWS, PK_ID : PK_ID + ROWS] = np.eye(ROWS, dtype=np.float32)
        pack[0:E, PK_CT : PK_CT + ROWS] = C[rs].T
        in_maps.append(
            {
                "pack": pack.astype(np.float16),
                "biasd": Bb,
                "wpair": wpair,
                "scl": scl,
            }
        )
    return in_maps


def run(inputs, mode=MODE, trace=False):
    """Returns (output [512,512] fp32, BassKernelResults)."""
    from concourse.bass_utils import run_bass_kernel_spmd

    nc = _get_nc(mode)
    in_maps = _prep_in_maps(inputs, mode)
    res = run_bass_kernel_spmd(nc, in_maps, list(range(NCORES)), trace=trace)
    out = np.concatenate([r["out"] for r in res.results], axis=0)
    return out, res


def kernel(**inputs) -> np.ndarray:
    out, _ = run(inputs)
    return out


# revision 20
# speedup vs baseline: 3.6649x; 1.0150x over previous
"""Blended-expert MLP (MoE routing) Trainium2 Bass kernel.

Math: reference computes, per layer,
    h = elu( einsum("bi,bio->bo", x, einsum("be,eio->bio", c, w)) + c @ b )
which factorizes as
    h = elu( sum_e (c[:,e] * x) @ W_e  +  c @ b )
(row-scaling commutes with the matmul), so per layer we scale X^T by
c_e on the vector engine and run 8 [rows,512]x[512,512] matmuls plus
one tiny K=8 matmul for the blended bias, ALL accumulating into a
single PSUM tile. Then ELU, then a PE transpose to produce the next
layer's stationary operand.

Sharding: data-parallel over the batch. B=512 rows split across 8
NeuronCores (64 rows each); the expert weights are replicated to every
core. No collectives (on-chip AllReduce has a ~20us latency floor).

int8 weight path (the big lever vs the fp16 baseline): the weight DMA
is the wall (fp16 = 12.6 MB/core at ~358 GB/s/core = 35 us). Weights
are uniform-distributed, so int8 quantization costs only ~0.4%%/layer
(measured 6.8e-3 total rel err vs the 2e-2 budget) and halves the DMA
to 6.3 MB (~18.6 us). The PE has no int8 mode, so weights are widened
on-chip to EXACT fp16 integers in [-127,127]:
  - DVE chunks ride as uint16 byte-pairs (host packs col j of experts
    0-3 into the low byte, col j of experts 4-7 into the high byte);
    two tensor_scalar ops split them: (v & 0xFF) - 128 and
    (v >> 8) - 128. All operands 2-byte -> DVE 4x mode, ~1.2us/chunk.
  - ACT / GPSIMD chunks ride as plain uint8; one activation
    Copy(in - 128) / tensor_scalar subtract per half, ~3.5us/chunk.
  Assignment per layer: k=0,3 -> DVE (layer closers need low latency),
  k=1 -> GPSIMD, k=2 -> ACT.
The float scale s_w (per layer, max|w|/127) is applied at PSUM
evacuation time via a per-partition fp32 scale AP (runtime data, so
the NEFF needs no recompile if inputs change); biases are pre-divided
by s_w on the host (they are structurally zero in this problem).

fp16 x/matmul path unchanged from the baseline: ~7e-4 of the error
budget. PE warmup matmuls get the HAM clock gate to 2.4 GHz before
real work; k-outer matmul order fires each chunk's matmuls on widen
completion; even/odd experts run concurrently in the two column
halves of the PE array.
"""

import numpy as np

B, E, D = 512, 8, 512
NCORES = 8
ROWS = B // NCORES  # 64
KC = D // 128  # 4 contraction chunks of 128
NCHUNK = 3 * KC  # 12

# pack tensor column layout (per 128 partitions)
PK_XT = 0  # [128, 256]: layer-1 x^T chunk k at cols [64k, 64k+64)
PK_CB = 256  # [128, 1024]: c broadcast; col 128e+64j+b = C[b,e], all partitions
PK_ID = PK_CB + E * 2 * ROWS  # [64, 64]: identity, partitions 0..63
PK_CT = PK_ID + ROWS  # [8, 64]: coef^T, partitions 0..7
PCK = PK_CT + ROWS

MODE = "i8"
N_WARMUP = 7

# every chunk rides as uint16 byte-pairs: lo byte = experts 0-3, hi byte =
# experts 4-7. DVE extracts the hi half via v/256-128 (693ns, 4x mode); the
# lo half is a stride-2 uint8 subtract: DVE 1.23us / ACT 2.0us measured.
# GPSIMD is catastrophic on integer ops (29us!) but fine on fp16 (800ns per
# [128,256] mult), so it takes most of the xs scaling instead.
LO_ENG = {0: "dve", 1: "act", 2: "act", 3: "act"}
# matmul issue order: hi-half experts first (their extract lands first)
ORD_E = [4, 5, 6, 7, 0, 1, 2, 3]
# xs scale ops: layers 1-2 keep the first-consumed experts on the fast DVE
DVE_SCALE_E = (4, 5, 6)

_NC_CACHE = {}


def _build(mode):
    from contextlib import ExitStack

    import concourse.bacc as bacc
    import concourse.mybir as mybir
    import concourse.tile as tile

    f32 = mybir.dt.float32
    f16 = mybir.dt.float16
    u16 = mybir.dt.uint16
    u8 = mybir.dt.uint8
    Alu = mybir.AluOpType
    Act = mybir.ActivationFunctionType

    nc = bacc.Bacc()
    pack_d = nc.declare_dram_parameter("pack", [128, PCK], f16, isOutput=False)
    scl_d = nc.declare_dram_parameter("scl", [128, 4], f32, isOutput=False)
    bias_d = nc.declare_dram_parameter("biasd", [E, 3 * D], f16, isOutput=False)
    wpair_d = nc.declare_dram_parameter(
        "wpair", [NCHUNK, 128, E * D // 2], u16, isOutput=False
    )
    out_d = nc.declare_dram_parameter("out", [ROWS, D], f32, isOutput=True)

    HED = E * D // 2  # 2048: wfp column offset of experts 4-7

    with ExitStack() as ctx:
        tc = ctx.enter_context(tile.TileContext(nc))
        const = ctx.enter_context(tc.tile_pool(name="const", bufs=1))
        # all 12 pair chunks live simultaneously: no WAR gating of the DMA
        # stream behind widen consumption (bufs=4 stretched the DMA window
        # from 18.6us to 32us)
        prpool = ctx.enter_context(tc.tile_pool(name="prp", bufs=NCHUNK))
        wfpool = ctx.enter_context(tc.tile_pool(name="wfp", bufs=6))
        spool = ctx.enter_context(tc.tile_pool(name="sp", bufs=24))
        hpool = ctx.enter_context(tc.tile_pool(name="hp", bufs=2))
        xpool = ctx.enter_context(tc.tile_pool(name="xp", bufs=2))
        acc_ps = ctx.enter_context(tc.tile_pool(name="acc", bufs=3, space="PSUM"))
        pt_ps = ctx.enter_context(tc.tile_pool(name="pt", bufs=3, space="PSUM"))
        wm_ps = ctx.enter_context(tc.tile_pool(name="wm", bufs=1, space="PSUM"))

        # PE warmup: garbage matmuls on a zeroed tile (output never read) so
        # the HAM clock gate reaches 2.4 GHz before the first real matmul.
        # memset on DVE (ready ~3.3us) rather than gpsimd (ready later).
        warm = const.tile([128, ROWS + D], f16)
        nc.vector.memset(warm[:], 0.0)
        wps = wm_ps.tile([ROWS, D], f32, tag="warm")
        for _ in range(N_WARMUP):
            nc.tensor.matmul(
                wps[:], warm[:, 0:ROWS], warm[:, ROWS:], start=True, stop=True
            )

        scl_t = const.tile([128, 4], f32)
        nc.sync.dma_start(scl_t[:], scl_d[:])
        pack_t = const.tile([128, PCK], f16)
        nc.sync.dma_start(pack_t[:], pack_d[:])
        bias_t = const.tile([E, 3 * D], f16)
        nc.gpsimd.dma_start(bias_t[:], bias_d[:])

        coeft_ap = pack_t[0:E, PK_CT : PK_CT + ROWS]
        ident_ap = pack_t[0:ROWS, PK_ID : PK_ID + ROWS]
        xt_tile, xt_off = pack_t, PK_XT  # current x^T source: [128, 256] at offset

        # all weight-chunk DMAs up-front on the sync queue in consumption
        # order; the HWDGE lane round-robin paces them at full bandwidth
        raw_tiles = []
        for c in range(NCHUNK):
            wt = prpool.tile([128, HED], u16, tag="wp")
            if c == NCHUNK - 1:
                # split the final chunk so experts 0-1/4-5 land earlier and
                # only the tail experts gate on the very last transfer
                nc.sync.dma_start(
                    wt[:, 0 : HED // 2], wpair_d[c, :, 0 : HED // 2]
                )
                nc.sync.dma_start(wt[:, HED // 2 :], wpair_d[c, :, HED // 2 :])
            else:
                nc.sync.dma_start(wt[:], wpair_d[c, :, :])
            raw_tiles.append(wt)

        def widen(c):
            """Emit widen ops for chunk c; returns the fp16 weight tile."""
            wt = raw_tiles[c]
            wf = wfpool.tile([128, E * D], f16, tag="wf")
            # hi extract on DVE, all-arith (the ISA has no DVE mod/bitwise
            # mixing): v/256 - 128 = w_q_hi + lo_byte/256; the host pre-
            # compensates that leakage into the hi byte.
            nsplit = 2 if c == NCHUNK - 1 else 1
            step = HED // nsplit
            for s in range(nsplit):
                lo, hi = s * step, (s + 1) * step
                nc.vector.tensor_scalar(
                    wf[:, HED + lo : HED + hi], wt[:, lo:hi],
                    1.0 / 256.0, 128.0, Alu.mult, Alu.subtract,
                )
            # lo extract: stride-2 uint8 view minus 128, on ACT or DVE
            lov = wt[:].bitcast(u8).rearrange("p (n two) -> p two n", two=2)
            for s in range(nsplit):
                lo, hi = s * step, (s + 1) * step
                if LO_ENG[c % KC] == "act":
                    nc.scalar.activation(
                        wf[:, lo:hi], lov[:, 0, lo:hi], Act.Copy, bias=-128.0
                    )
                else:
                    nc.vector.tensor_scalar(
                        wf[:, lo:hi], lov[:, 0, lo:hi], 128.0, None,
                        Alu.subtract,
                    )
            return wf

        for layer in range(3):
            # widen this layer's chunks (emitted per layer so queued engine
            # work stays roughly in execution order)
            wfs = [widen(layer * KC + k) for k in range(KC)]
            sw_ap = scl_t[0:ROWS, layer : layer + 1]

            # scale x^T by c_e along the batch (free) dim, one full-width op
            # per expert, emitted in consumption order. Layer 0 scales only
            # need pack, so the otherwise-idle GPSIMD does them all; later
            # layers keep the first-consumed experts on the faster DVE.
            scaled = [None] * E
            for e in ORD_E:
                sc = spool.tile([128, KC * ROWS], f16, tag="sc")
                cbs = pack_t[
                    :, PK_CB + 2 * ROWS * e : PK_CB + 2 * ROWS * (e + 1)
                ]
                if layer > 0 and e in DVE_SCALE_E:
                    # DVE: two contiguous half ops keep the 2x mode (a
                    # stride-0 broadcast AP drops it to 1x)
                    for half in range(2):
                        lo, hi = 2 * ROWS * half, 2 * ROWS * (half + 1)
                        nc.vector.tensor_tensor(
                            out=sc[:, lo:hi],
                            in0=xt_tile[:, xt_off + lo : xt_off + hi],
                            in1=cbs,
                            op=Alu.mult,
                        )
                else:
                    # GPSIMD: one full-width op (big per-op overhead there)
                    nc.gpsimd.tensor_tensor(
                        out=sc[:].rearrange("p (r c) -> p r c", r=2),
                        in0=xt_tile[
                            :, xt_off : xt_off + KC * ROWS
                        ].rearrange("p (r c) -> p r c", r=2),
                        in1=cbs.unsqueeze(1).broadcast_to((128, 2, 2 * ROWS)),
                        op=Alu.mult,
                    )
                scaled[e] = sc

            # one accumulation group: 32 expert matmuls + bias matmul (K=8).
            # k-outer order: each chunk's 8 expert matmuls fire as soon as
            # its widen lands. Even/odd experts run CONCURRENTLY in the two
            # column halves of the PE array (tile_position); the partition
            # halves of acc are summed afterwards.
            acc = acc_ps.tile([2 * ROWS, D], f32, tag="acc")
            nc.tensor.matmul(
                acc[0:ROWS, :],
                coeft_ap,
                bias_t[:, D * layer : D * (layer + 1)],
                start=True,
                stop=False,
                tile_position=(0, 0),
                skip_group_check=True,
            )
            for k in range(KC):
                for e in ORD_E:
                    half = e % 2
                    nc.tensor.matmul(
                        acc[half * ROWS : (half + 1) * ROWS, :],
                        scaled[e][:, ROWS * k : ROWS * (k + 1)],
                        wfs[k][:, D * e : D * (e + 1)],
                        start=(k == 0 and e == ORD_E[1]),
                        stop=(k == KC - 1 and e in ORD_E[-2:]),
                        tile_position=(0, half * ROWS),
                        skip_group_check=True,
                    )

            # evacuate even half with the s_w scale (ACT), merge+scale the
            # odd half (DVE stt), elu, transpose; pipelined per 128-column
            # quarter so each quarter flows through the chain independently
            t0 = hpool.tile([ROWS, D], f32, tag="t0")
            hpre = hpool.tile([ROWS, D], f32, tag="hpre")
            HD = D // 2
            if layer < 2:
                ex = hpool.tile([ROWS, D], f32, tag="ex")
                h = hpool.tile([ROWS, D], f16, tag="h")
                xt_t = xpool.tile([128, KC * ROWS], f16, tag="xt")
                # merge/elu per 256-col half (fewer, bigger DVE/ACT ops);
                # transpose + evacuation still per 128-col quarter
                for hh in range(2):
                    hs = slice(HD * hh, HD * (hh + 1))
                    nc.scalar.activation(
                        t0[:, hs], acc[0:ROWS, hs], Act.Copy, scale=sw_ap
                    )
                    nc.vector.scalar_tensor_tensor(
                        out=hpre[:, hs],
                        in0=acc[ROWS:, hs],
                        scalar=sw_ap,
                        in1=t0[:, hs],
                        op0=Alu.mult,
                        op1=Alu.add,
                    )
                    nc.scalar.activation(ex[:, hs], hpre[:, hs], Act.Exp)
                    nc.vector.tensor_scalar(
                        ex[:, hs], ex[:, hs], 1.0, 0.0, Alu.subtract, Alu.min
                    )
                    nc.vector.scalar_tensor_tensor(
                        out=h[:, hs],
                        in0=hpre[:, hs],
                        scalar=0.0,
                        in1=ex[:, hs],
                        op0=Alu.max,
                        op1=Alu.add,
                    )
                    for q in (2 * hh, 2 * hh + 1):
                        qs = slice(128 * q, 128 * (q + 1))
                        pt = pt_ps.tile([128, ROWS], f16, tag="pt")
                        nc.tensor.transpose(pt[:], h[:, qs], ident_ap)
                        dst = xt_t[:, ROWS * q : ROWS * (q + 1)]
                        if q % 2 == 0:
                            nc.scalar.copy(dst, pt[:])
                        else:
                            nc.vector.tensor_copy(dst, pt[:])
                xt_tile, xt_off = xt_t, 0
            else:
                # stream the output per column half, right behind the merge
                for cc in range(2):
                    cs = slice(HD * cc, HD * (cc + 1))
                    nc.scalar.activation(
                        t0[:, cs], acc[0:ROWS, cs], Act.Copy, scale=sw_ap
                    )
                    nc.vector.scalar_tensor_tensor(
                        out=hpre[:, cs],
                        in0=acc[ROWS:, cs],
                        scalar=sw_ap,
                        in1=t0[:, cs],
                        op0=Alu.mult,
                        op1=Alu.add,
                    )
                    nc.sync.dma_start(out_d[:, cs], hpre[:, cs])

    nc.compile()
    return nc


def _get_nc(mode=MODE):
    if mode not in _NC_CACHE:
        _NC_CACHE[mode] = _build(mode)
    return _NC_CACHE[mode]


def _prep_in_maps(inputs, mode=MODE):
    X = np.asarray(inputs["X"], np.float32)
    C = np.asarray(inputs["blending_coef"], np.float32)
    ws = [np.asarray(inputs[f"w_l{i}"], np.float32) for i in (1, 2, 3)]
    bs = [np.asarray(inputs[f"b_l{i}"], np.float32) for i in (1, 2, 3)]

    # W[l][i, e*D+o] = w_l[e, i, o]; int8-quantize per layer
    sw = np.array([max(np.abs(w).max() / 127.0, 1e-30) for w in ws], np.float32)
    scaled_ws = []
    for l, w in enumerate(ws):
        W = w.transpose(1, 0, 2).reshape(D, E * D)
        scaled_ws.append((W / sw[l]).astype(np.float64))  # in [-127, 127]
    wpair = np.zeros((NCHUNK, 128, E * D // 2), np.uint16)
    HED = E * D // 2
    for c in range(NCHUNK):
        l, k = c // KC, c % KC
        sub = scaled_ws[l][128 * k : 128 * (k + 1)]
        # pair layout: lo byte = experts 0-3, hi byte = experts 4-7. The
        # on-chip hi extract is v/256 - 128 = w_hi + lo/256, so pre-
        # subtract the known lo/256 leakage before rounding.
        a = (np.round(sub[:, :HED]).clip(-127, 127) + 128.0).astype(np.uint16)
        b = np.round(sub[:, HED:] + 128.0 - a / 256.0).clip(0, 255)
        wpair[c] = a | (b.astype(np.uint16) << 8)

    Bb = np.concatenate([b / s for b, s in zip(bs, sw)], axis=1).astype(
        np.float16
    )  # [E, 3*D], pre-divided by s_w (zeros in this problem)
    scl = np.broadcast_to(
        np.concatenate([sw, [1.0]]).astype(np.float32), (128, 4)
    ).copy()

    in_maps = []
    for c in range(NCORES):
        rs = slice(c * ROWS, (c + 1) * ROWS)
        pack = np.zeros((128, PCK), np.float32)
        # xt chunks: pack[p, 64k+b] = X[rows][b, 128k+p]
        xt = np.ascontiguousarray(X[rs].T)  # [512, 64]
        pack[:, PK_XT : PK_XT + KC * ROWS] = (
            xt.reshape(KC, 128, ROWS).transpose(1, 0, 2).reshape(128, KC * ROWS)
        )
        # c broadcast: pack[p, PK_CB + 128e + 64j + b] = C[rs][b, e]
        pack[:, PK_CB : PK_CB + E * 2 * ROWS] = np.broadcast_to(
            C[rs].T[:, None, :], (E, 2, ROWS)
        ).reshape(1, E * 2 * ROWS)
        pack[0:ROWS, PK_ID : PK_ID + ROWS] = np.eye(ROWS, dtype=np.float32)
        pack[0:E, PK_CT : PK_CT + ROWS] = C[rs].T
        in_maps.append(
            {
                "pack": pack.astype(np.float16),
                "biasd": Bb,
                "wpair": wpair,
                "scl": scl,
            }
        )
    return in_maps


def run(inputs, mode=MODE, trace=False):
    """Returns (output [512,512] fp32, BassKernelResults)."""
    from concourse.bass_utils import run_bass_kernel_spmd

    nc = _get_nc(mode)
    in_maps = _prep_in_maps(inputs, mode)
    res = run_bass_kernel_spmd(nc, in_maps, list(range(NCORES)), trace=trace)
    out = np.concatenate([r["out"] for r in res.results], axis=0)
    return out, res


def kernel(**inputs) -> np.ndarray:
    out, _ = run(inputs)
    return out


# revision 34
# speedup vs baseline: 3.8216x; 1.0428x over previous
"""Blended-expert MLP (MoE routing) Trainium2 Bass kernel.

Math: reference computes, per layer,
    h = elu( einsum("bi,bio->bo", x, einsum("be,eio->bio", c, w)) + c @ b )
which factorizes as
    h = elu( sum_e (c[:,e] * x) @ W_e  +  c @ b )
(row-scaling commutes with the matmul), so per layer we scale X^T by
c_e on the vector engine and run 8 [rows,512]x[512,512] matmuls plus
one tiny K=8 matmul for the blended bias, ALL accumulating into a
single PSUM tile. Then ELU, then a PE transpose to produce the next
layer's stationary operand.

Sharding: data-parallel over the batch. B=512 rows split across 8
NeuronCores (64 rows each); the expert weights are replicated to every
core. No collectives (on-chip AllReduce has a ~20us latency floor).

int8 weight path (the big lever vs the fp16 baseline): the weight DMA
is the wall (fp16 = 12.6 MB/core at ~358 GB/s/core = 35 us). Weights
are uniform-distributed, so int8 quantization costs only ~0.4%%/layer
(measured 6.8e-3 total rel err vs the 2e-2 budget) and halves the DMA
to 6.3 MB (~18.6 us). The PE has no int8 mode, so weights are widened
on-chip to EXACT fp16 integers in [-127,127]:
  - DVE chunks ride as uint16 byte-pairs (host packs col j of experts
    0-3 into the low byte, col j of experts 4-7 into the high byte);
    two tensor_scalar ops split them: (v & 0xFF) - 128 and
    (v >> 8) - 128. All operands 2-byte -> DVE 4x mode, ~1.2us/chunk.
  - ACT / GPSIMD chunks ride as plain uint8; one activation
    Copy(in - 128) / tensor_scalar subtract per half, ~3.5us/chunk.
  Assignment per layer: k=0,3 -> DVE (layer closers need low latency),
  k=1 -> GPSIMD, k=2 -> ACT.
The float scale s_w (per layer, max|w|/127) is applied at PSUM
evacuation time via a per-partition fp32 scale AP (runtime data, so
the NEFF needs no recompile if inputs change); biases are pre-divided
by s_w on the host (they are structurally zero in this problem).

fp16 x/matmul path unchanged from the baseline: ~7e-4 of the error
budget. PE warmup matmuls get the HAM clock gate to 2.4 GHz before
real work; k-outer matmul order fires each chunk's matmuls on widen
completion; even/odd experts run concurrently in the two column
halves of the PE array.
"""

import numpy as np

B, E, D = 512, 8, 512
NCORES = 8
ROWS = B // NCORES  # 64
KC = D // 128  # 4 contraction chunks of 128
NCHUNK = 3 * KC  # 12

# pack tensor column layout (per 128 partitions)
PK_XT = 0  # [128, 256]: layer-1 x^T chunk k at cols [64k, 64k+64)
PK_CB = 256  # [128, 2048]: c broadcast; col 256e+64j+b = C[b,e], all partitions
PK_ID = PK_CB + E * KC * ROWS  # [64, 64]: identity, partitions 0..63
PK_CT = PK_ID + ROWS  # [8, 64]: coef^T, partitions 0..7
PCK = PK_CT + ROWS

MODE = "i8"
N_WARMUP = 7

# every chunk rides as uint16 byte-pairs: lo byte = experts 0-3, hi byte =
# experts 4-7. DVE extracts the hi half via v/256-128 (693ns, 4x mode); the
# lo half is a stride-2 uint8 subtract: DVE 1.23us / ACT 2.0us measured.
# GPSIMD is catastrophic on integer ops (29us!) but fine on fp16/fp32 SBUF
# ops, so it takes the xs scaling and part of the elu chain instead.
LO_DVE = {0, 1, 4, 5, 8}  # chunks whose lo extract runs on DVE (rest ACT)
# matmul issue order: hi-half experts first (their extract lands first)
ORD_E = [4, 5, 6, 7, 0, 1, 2, 3]
# xs scale ops, layers 1-2: first-consumed experts on the fast DVE
DVE_SCALE_E = (4, 5, 6, 7)
N_KEEPWARM = 8

_NC_CACHE = {}


def _build(mode):
    from contextlib import ExitStack

    import concourse.bacc as bacc
    import concourse.mybir as mybir
    import concourse.tile as tile

    f32 = mybir.dt.float32
    f16 = mybir.dt.float16
    u16 = mybir.dt.uint16
    u8 = mybir.dt.uint8
    Alu = mybir.AluOpType
    Act = mybir.ActivationFunctionType

    nc = bacc.Bacc()
    pack_d = nc.declare_dram_parameter("pack", [128, PCK], f16, isOutput=False)
    scl_d = nc.declare_dram_parameter("scl", [128, 4], f32, isOutput=False)
    bias_d = nc.declare_dram_parameter("biasd", [E, 3 * D], f16, isOutput=False)
    wpair_d = nc.declare_dram_parameter(
        "wpair", [NCHUNK, 128, E * D // 2], u16, isOutput=False
    )
    out_d = nc.declare_dram_parameter("out", [ROWS, D], f32, isOutput=True)

    HED = E * D // 2  # 2048: wfp column offset of experts 4-7

    with ExitStack() as ctx:
        tc = ctx.enter_context(tile.TileContext(nc))
        const = ctx.enter_context(tc.tile_pool(name="const", bufs=1))
        # all 12 pair chunks live simultaneously: no WAR gating of the DMA
        # stream behind widen consumption (bufs=4 stretched the DMA window
        # from 18.6us to 32us)
        prpool = ctx.enter_context(tc.tile_pool(name="prp", bufs=NCHUNK))
        wfpool = ctx.enter_context(tc.tile_pool(name="wfp", bufs=6))
        spool = ctx.enter_context(tc.tile_pool(name="sp", bufs=18))
        s0pool = ctx.enter_context(tc.tile_pool(name="s0", bufs=1))
        hpool = ctx.enter_context(tc.tile_pool(name="hp", bufs=2))
        xpool = ctx.enter_context(tc.tile_pool(name="xp", bufs=2))
        acc_ps = ctx.enter_context(tc.tile_pool(name="acc", bufs=3, space="PSUM"))
        pt_ps = ctx.enter_context(tc.tile_pool(name="pt", bufs=3, space="PSUM"))
        wm_ps = ctx.enter_context(tc.tile_pool(name="wm", bufs=1, space="PSUM"))

        # PE warmup: garbage matmuls on a zeroed tile (output never read) so
        # the HAM clock gate reaches 2.4 GHz before the first real matmul.
        # memset on DVE (ready ~3.3us) rather than gpsimd (ready later).
        warm = const.tile([128, ROWS + D], f16)
        nc.vector.memset(warm[:], 0.0)
        wps = wm_ps.tile([ROWS, D], f32, tag="warm")
        for _ in range(N_WARMUP):
            nc.tensor.matmul(
                wps[:], warm[:, 0:ROWS], warm[:, ROWS:], start=True, stop=True
            )

        scl_t = const.tile([128, 4], f32)
        nc.sync.dma_start(scl_t[:], scl_d[:])
        pack_t = const.tile([128, PCK], f16)
        nc.sync.dma_start(pack_t[:], pack_d[:])
        bias_t = const.tile([E, 3 * D], f16)
        nc.gpsimd.dma_start(bias_t[:], bias_d[:])

        coeft_ap = pack_t[0:E, PK_CT : PK_CT + ROWS]
        ident_ap = pack_t[0:ROWS, PK_ID : PK_ID + ROWS]
        xt_tile, xt_off = pack_t, PK_XT  # current x^T source: [128, 256] at offset

        # all weight-chunk DMAs up-front on the sync queue in consumption
        # order; the HWDGE lane round-robin paces them at full bandwidth
        raw_tiles = []
        for c in range(NCHUNK):
            wt = prpool.tile([128, HED], u16, tag="wp")
            if c == NCHUNK - 1:
                # split the final chunk so experts 0-1/4-5 land earlier and
                # only the tail experts gate on the very last transfer
                nc.sync.dma_start(
                    wt[:, 0 : HED // 2], wpair_d[c, :, 0 : HED // 2]
                )
                nc.sync.dma_start(wt[:, HED // 2 :], wpair_d[c, :, HED // 2 :])
            else:
                nc.sync.dma_start(wt[:], wpair_d[c, :, :])
            raw_tiles.append(wt)

        def widen(c):
            """Emit widen ops for chunk c; returns the fp16 weight tile."""
            wt = raw_tiles[c]
            wf = wfpool.tile([128, E * D], f16, tag="wf")
            # hi extract on DVE, all-arith (the ISA has no DVE mod/bitwise
            # mixing): v/256 - 128 = w_q_hi + lo_byte/256; the host pre-
            # compensates that leakage into the hi byte.
            nsplit = 2 if c == NCHUNK - 1 else 1
            step = HED // nsplit
            for s in range(nsplit):
                lo, hi = s * step, (s + 1) * step
                nc.vector.tensor_scalar(
                    wf[:, HED + lo : HED + hi], wt[:, lo:hi],
                    1.0 / 256.0, 128.0, Alu.mult, Alu.subtract,
                )
            # lo extract: stride-2 uint8 view minus 128, on ACT or DVE
            lov = wt[:].bitcast(u8).rearrange("p (n two) -> p two n", two=2)
            for s in range(nsplit):
                lo, hi = s * step, (s + 1) * step
                if c in LO_DVE:
                    nc.vector.tensor_scalar(
                        wf[:, lo:hi], lov[:, 0, lo:hi], 128.0, None,
                        Alu.subtract,
                    )
                else:
                    nc.scalar.activation(
                        wf[:, lo:hi], lov[:, 0, lo:hi], Act.Copy, bias=-128.0
                    )
            return wf

        for layer in range(3):
            # widen this layer's chunks (emitted per layer so queued engine
            # work stays roughly in execution order)
            wfs = [widen(layer * KC + k) for k in range(KC)]
            sw_ap = scl_t[0:ROWS, layer : layer + 1]

            # scale x^T by c_e along the batch (free) dim, one full-width op
            # per expert, emitted in consumption order. Layer 0 scales only
            # need pack, so the otherwise-idle GPSIMD does them all; later
            # layers keep the first-consumed experts on the faster DVE.
            # xs = x^T * c_e. The c-broadcast is stored 4x-replicated, so
            # every op is contiguous. Layer 0 only needs pack: one GPSIMD
            # op covers all 8 experts. Later layers split per expert:
            # first-consumed experts on DVE (351ns), the rest on GPSIMD.
            SCW = KC * ROWS  # 256
            xt_ap = xt_tile[:, xt_off : xt_off + SCW]
            if layer == 0:
                sc_all = s0pool.tile([128, E * SCW], f16, tag="sc0")
                nc.gpsimd.tensor_tensor(
                    out=sc_all[:].rearrange("p (e c) -> p e c", e=E),
                    in0=xt_ap.unsqueeze(1).broadcast_to((128, E, SCW)),
                    in1=pack_t[:, PK_CB : PK_CB + E * SCW].rearrange(
                        "p (e c) -> p e c", e=E
                    ),
                    op=Alu.mult,
                )
                scaled = [(sc_all, e * SCW) for e in range(E)]
            else:
                scaled = [None] * E
                for e in ORD_E:
                    sc = spool.tile([128, SCW], f16, tag="sc")
                    eng = (
                        nc.vector if e in DVE_SCALE_E else nc.gpsimd
                    )
                    eng.tensor_tensor(
                        out=sc[:],
                        in0=xt_ap,
                        in1=pack_t[:, PK_CB + SCW * e : PK_CB + SCW * (e + 1)],
                        op=Alu.mult,
                    )
                    scaled[e] = (sc, 0)

            # one accumulation group: 32 expert matmuls + bias matmul (K=8).
            # k-outer order: each chunk's 8 expert matmuls fire as soon as
            # its widen lands. Even/odd experts run CONCURRENTLY in the two
            # column halves of the PE array (tile_position); the partition
            # halves of acc are summed afterwards.
            acc = acc_ps.tile([2 * ROWS, D], f32, tag="acc")
            nc.tensor.matmul(
                acc[0:ROWS, :],
                coeft_ap,
                bias_t[:, D * layer : D * (layer + 1)],
                start=True,
                stop=False,
                tile_position=(0, 0),
                skip_group_check=True,
            )
            for k in range(KC):
                for e in ORD_E:
                    half = e % 2
                    sct, sco = scaled[e]
                    nc.tensor.matmul(
                        acc[half * ROWS : (half + 1) * ROWS, :],
                        sct[:, sco + ROWS * k : sco + ROWS * (k + 1)],
                        wfs[k][:, D * e : D * (e + 1)],
                        start=(k == 0 and e == ORD_E[1]),
                        stop=(k == KC - 1 and e in ORD_E[-2:]),
                        tile_position=(0, half * ROWS),
                        skip_group_check=True,
                    )

            # evacuate even half with the s_w scale (ACT), merge+scale the
            # odd half (DVE stt), elu, transpose; pipelined per 128-column
            # quarter so each quarter flows through the chain independently
            t0 = hpool.tile([ROWS, D], f32, tag="t0")
            hpre = hpool.tile([ROWS, D], f32, tag="hpre")
            HD = D // 2
            if layer < 2:
                # keep the PE clock at 2.4 GHz across the boundary idle
                # (an idle PE re-throttles and the next layer's matmuls
                # run 2-3x slower)
                for _ in range(N_KEEPWARM):
                    nc.tensor.matmul(
                        wps[:], warm[:, 0:ROWS], warm[:, ROWS:],
                        start=True, stop=True,
                    )
                ex = hpool.tile([ROWS, D], f32, tag="ex")
                h = hpool.tile([ROWS, D], f16, tag="h")
                xt_t = xpool.tile([128, KC * ROWS], f16, tag="xt")
                # full-width merge/elu chain (per-op overhead dominates
                # smaller slices); min and h land on GPSIMD (SBUF-only
                # fp32 ops are fine there); transposes per 128-quarter
                nc.scalar.activation(
                    t0[:], acc[0:ROWS, :], Act.Copy, scale=sw_ap
                )
                nc.vector.scalar_tensor_tensor(
                    out=hpre[:],
                    in0=acc[ROWS:, :],
                    scalar=sw_ap,
                    in1=t0[:],
                    op0=Alu.mult,
                    op1=Alu.add,
                )
                nc.scalar.activation(ex[:], hpre[:], Act.Exp)
                nc.vector.tensor_scalar(
                    ex[:], ex[:], 1.0, 0.0, Alu.subtract, Alu.min
                )
                nc.vector.scalar_tensor_tensor(
                    out=h[:],
                    in0=hpre[:],
                    scalar=0.0,
                    in1=ex[:],
                    op0=Alu.max,
                    op1=Alu.add,
                )
                for q in range(KC):
                    qs = slice(128 * q, 128 * (q + 1))
                    pt = pt_ps.tile([128, ROWS], f16, tag="pt")
                    nc.tensor.transpose(pt[:], h[:, qs], ident_ap)
                    dst = xt_t[:, ROWS * q : ROWS * (q + 1)]
                    if q % 2 == 0:
                        nc.scalar.copy(dst, pt[:])
                    else:
                        nc.vector.tensor_copy(dst, pt[:])
                xt_tile, xt_off = xt_t, 0
            else:
                # stream the output per column half, right behind the merge
                for cc in range(2):
                    cs = slice(HD * cc, HD * (cc + 1))
                    nc.scalar.activation(
                        t0[:, cs], acc[0:ROWS, cs], Act.Copy, scale=sw_ap
                    )
                    nc.vector.scalar_tensor_tensor(
                        out=hpre[:, cs],
                        in0=acc[ROWS:, cs],
                        scalar=sw_ap,
                        in1=t0[:, cs],
                        op0=Alu.mult,
                        op1=Alu.add,
                    )
                    nc.sync.dma_start(out_d[:, cs], hpre[:, cs])

    nc.compile()
    return nc


def _get_nc(mode=MODE):
    if mode not in _NC_CACHE:
        _NC_CACHE[mode] = _build(mode)
    return _NC_CACHE[mode]


def _prep_in_maps(inputs, mode=MODE):
    X = np.asarray(inputs["X"], np.float32)
    C = np.asarray(inputs["blending_coef"], np.float32)
    ws = [np.asarray(inputs[f"w_l{i}"], np.float32) for i in (1, 2, 3)]
    bs = [np.asarray(inputs[f"b_l{i}"], np.float32) for i in (1, 2, 3)]

    # W[l][i, e*D+o] = w_l[e, i, o]; int8-quantize per layer
    sw = np.array([max(np.abs(w).max() / 127.0, 1e-30) for w in ws], np.float32)
    scaled_ws = []
    for l, w in enumerate(ws):
        W = w.transpose(1, 0, 2).reshape(D, E * D)
        scaled_ws.append((W / sw[l]).astype(np.float64))  # in [-127, 127]
    wpair = np.zeros((NCHUNK, 128, E * D // 2), np.uint16)
    HED = E * D // 2
    for c in range(NCHUNK):
        l, k = c // KC, c % KC
        sub = scaled_ws[l][128 * k : 128 * (k + 1)]
        # pair layout: lo byte = experts 0-3, hi byte = experts 4-7. The
        # on-chip hi extract is v/256 - 128 = w_hi + lo/256, so pre-
        # subtract the known lo/256 leakage before rounding.
        a = (np.round(sub[:, :HED]).clip(-127, 127) + 128.0).astype(np.uint16)
        b = np.round(sub[:, HED:] + 128.0 - a / 256.0).clip(0, 255)
        wpair[c] = a | (b.astype(np.uint16) << 8)

    Bb = np.concatenate([b / s for b, s in zip(bs, sw)], axis=1).astype(
        np.float16
    )  # [E, 3*D], pre-divided by s_w (zeros in this problem)
    scl = np.broadcast_to(
        np.concatenate([sw, [1.0]]).astype(np.float32), (128, 4)
    ).copy()

    in_maps = []
    for c in range(NCORES):
        rs = slice(c * ROWS, (c + 1) * ROWS)
        pack = np.zeros((128, PCK), np.float32)
        # xt chunks: pack[p, 64k+b] = X[rows][b, 128k+p]
        xt = np.ascontiguousarray(X[rs].T)  # [512, 64]
        pack[:, PK_XT : PK_XT + KC * ROWS] = (
            xt.reshape(KC, 128, ROWS).transpose(1, 0, 2).reshape(128, KC * ROWS)
        )
        # c broadcast: pack[p, PK_CB + 256e + 64j + b] = C[rs][b, e]
        pack[:, PK_CB : PK_CB + E * KC * ROWS] = np.broadcast_to(
            C[rs].T[:, None, :], (E, KC, ROWS)
        ).reshape(1, E * KC * ROWS)
        pack[0:ROWS, PK_ID : PK_ID + ROWS] = np.eye(ROWS, dtype=np.float32)
        pack[0:E, PK_CT : PK_CT + ROWS] = C[rs].T
        in_maps.append(
            {
                "pack": pack.astype(np.float16),
                "biasd": Bb,
                "wpair": wpair,
                "scl": scl,
            }
        )
    return in_maps


def run(inputs, mode=MODE, trace=False):
    """Returns (output [512,512] fp32, BassKernelResults)."""
    from concourse.bass_utils import run_bass_kernel_spmd

    nc = _get_nc(mode)
    in_maps = _prep_in_maps(inputs, mode)
    res = run_bass_kernel_spmd(nc, in_maps, list(range(NCORES)), trace=trace)
    out = np.concatenate([r["out"] for r in res.results], axis=0)
    return out, res


def kernel(**inputs) -> np.ndarray:
    out, _ = run(inputs)
    return out
